# revision 5
# baseline (speedup 1.0000x reference)
"""Dropless MoE GLU-MLP kernel for 8 Trainium2 NeuronCores.

Strategy: expert-parallel. Host computes the routing (gates + per-expert
token lists), gathers each expert's tokens, and ships one expert per core.
Each core runs a 3-matmul GLU MLP over its (padded) token batch with all
matmul operands in fp16 (e5m10; the PE multiplies at FP22 and accumulates
fp32, so end-to-end error is ~5e-4 — measured against an fp64 oracle):

    AT = w1e @ Xe.T          [F, C]   (stationary = w1t chunks, moving = Xt)
    BT = v1e @ Xe.T          [F, C]
    GT = silu(AT) * BT       [F, C]   (ACT silu + DVE mul, PSUM-evicted)
    Y  = GT.T @ w2e          [C, H]   (stationary = GT chunks, moving = w2)

All matmuls use moving dim <=512 (one PSUM bank); at 512 they profile at
the 215.8 ns/MM issue floor and the body is PE-bound end to end, so the
optimization targets are the head (first real matmul issuing early, PE
never starved) and the tail (drain Y fast):

 - Per-core HBM read bandwidth is ~300 GB/s TOTAL across both HWDGE
   queues, so the head is fed by a single Sync-queue DMA stream whose
   issue order exactly matches PE consumption order: w1/v1 block 0, the
   eight j0-half x chunks (131 KB each, matching the 432 ns/chunk PE
   consumption rate), then the later w1/v1 blocks, the j1-half x chunks,
   and ft0's w2. No semaphore gating needed — queue order IS the
   bandwidth allocation. The first real matmul starts ~10 us in (vs ~16
   us for a monolithic xt load) and warmup matmuls keep the PE clock
   ramp continuous through the handoff (a PE idle gap costs ~6 us of
   half-rate re-ramp on top of the gap itself).
 - the mm1/mm2 k-loop interleaves the w1(pa) and v1(pb) accumulations so
   each x chunk is consumed at two matmuls per arrival.
 - later F-tiles' weights ride the Scalar HWDGE queue (its transfers land
   during the previous tile's combine phase when the Sync queue is idle),
   gated behind the previous tile's first combine matmul.
 - C pads the max per-expert token count to a multiple of 8 (not 128),
   shaving dead columns off mm1/mm2's moving dim; mm3's stationary
   chunks handle the ragged tail (<=128 wide costs the same instruction).
 - Y accumulates fp32 in SBUF across F-tiles and is emitted fp16 during
   the last F-tile as whole [P, H] rows (2 KB runs); the last row is
   split in half so the final DMA chases the final DVE add by ~0.4 us.

Host scatter-adds gate-scaled Y back to the full output.
"""

import numpy as np

import concourse.bass as bass
import concourse.tile as tile
from concourse import bacc, mybir
from concourse.bass_utils import run_bass_kernel_spmd
from concourse.tile import add_dep_helper

T, H, F, E, TOPK = 4096, 1024, 4096, 8, 2
P = 128
KH = H // P            # 8 k-chunks over the H contraction
FT = 512               # F tile width
NFT = F // FT          # 8 F tiles
KFT = FT // P          # 4 k-chunks per F tile in the combine matmul
HT = 512               # moving tile of H in the combine matmul
NHT = H // HT          # 2

_programs: dict[int, object] = {}


def _ntile_splits(C: int) -> list[tuple[int, int]]:
    """Split C into moving-dim tiles of <=512."""
    splits, off, rem = [], 0, C
    while rem > 0:
        take = min(512, rem)
        splits.append((off, take))
        off += take
        rem -= take
    return splits


def _build_program(C: int):
    f32 = mybir.dt.float32
    f16 = mybir.dt.float16
    MT = (C + P - 1) // P                # mm3 stationary chunks
    csz = [min(P, C - mt * P) for mt in range(MT)]
    nsplits = _ntile_splits(C)
    NJ = len(nsplits)

    nc = bacc.Bacc("TRN2", target_bir_lowering=False, debug=False, num_devices=E)
    # xt: [P, KH, C] — per-(k, j-half) slices are contiguous >=1KB runs
    xt_d = nc.dram_tensor("xt", [P, KH, C], f16, kind="ExternalInput").ap()
    # w1t/v1t: [P, NFT, KFT, KH, P] — one ft/block slice is a 2KB run
    w1_d = nc.dram_tensor("w1t", [P, NFT, KFT, KH, P], f16, kind="ExternalInput").ap()
    v1_d = nc.dram_tensor("v1t", [P, NFT, KFT, KH, P], f16, kind="ExternalInput").ap()
    w2_d = nc.dram_tensor("w2", [P, F // P, H], f16, kind="ExternalInput").ap()
    y_d = nc.dram_tensor("y", [P, MT, H], f16, kind="ExternalOutput").ap()

    with tile.TileContext(nc) as tc:
        with (
            tc.tile_pool(name="xk", bufs=1) as xk_pool,
            tc.tile_pool(name="yacc", bufs=1) as y_pool,
            tc.tile_pool(name="y16", bufs=2) as y16_pool,
            tc.tile_pool(name="w1f", bufs=2) as w1_pool,
            tc.tile_pool(name="v1f", bufs=2) as v1_pool,
            tc.tile_pool(name="w2f", bufs=2) as w2_pool,
            tc.tile_pool(name="gt", bufs=2) as g_pool,
            tc.tile_pool(name="sa", bufs=2) as a_pool,
            tc.tile_pool(name="wu", bufs=1) as wu_pool,
            tc.tile_pool(name="pa", bufs=2, space="PSUM") as pa_pool,
            tc.tile_pool(name="pb", bufs=2, space="PSUM") as pb_pool,
            tc.tile_pool(name="py", bufs=4, space="PSUM") as py_pool,
        ):
            # PE warmup during the initial DMA window: junk matmuls flip the
            # HAM clock gate to 8/8 and start the pstate ramp before the
            # first real matmul issues (~10.1us in).
            wu = wu_pool.tile([P, 512], f16)
            nc.vector.memset(wu[:], 0.0)
            wps = [pa_pool.tile([P, 512], f32, tag="pa", name="wp_a"),
                   pb_pool.tile([P, 512], f32, tag="pb", name="wp_b")]
            for i in range(7):
                nc.tensor.matmul(wps[i % 2][:], wu[:, :P], wu[:],
                                 start=True, stop=True)

            y_acc = y_pool.tile([P, MT, H], f32)

            # ft0 weight tiles (loaded on the Sync queue, interleaved with x)
            w1f0 = w1_pool.tile([P, KFT, KH, P], f16, name="w1f0")
            v1f0 = v1_pool.tile([P, KFT, KH, P], f16, name="v1f0")
            w2f0 = w2_pool.tile([P, KFT, H], f16, name="w2f0")

            # x chunk tiles, split by (j-half, k)
            xks = [[xk_pool.tile([P, nsz], f16, name=f"xk{j}_{k}",
                                 tag=f"xk{j}_{k}") for k in range(KH)]
                   for j, (noff, nsz) in enumerate(nsplits)]

            # ---- head DMA stream, in exact consumption order ----
            # The tile scheduler freely reorders independent DMAs across the
            # two HWDGE queues (and hoists later weight loads ahead of the x
            # chunks, starving the PE). Chain ordering-only deps so arrival
            # order matches consumption order; per-queue FIFO does the rest.
            head_dmas = []
            head_dmas.append(nc.sync.dma_start(w1f0[:, 0], w1_d[:, 0, 0]))
            head_dmas.append(nc.sync.dma_start(v1f0[:, 0], v1_d[:, 0, 0]))
            noff0, nsz0 = nsplits[0]
            for k in range(KH):
                head_dmas.append(
                    nc.sync.dma_start(xks[0][k][:], xt_d[:, k, noff0:noff0 + nsz0]))
            for q in range(1, KFT):
                head_dmas.append(nc.sync.dma_start(w1f0[:, q], w1_d[:, 0, q]))
                head_dmas.append(nc.sync.dma_start(v1f0[:, q], v1_d[:, 0, q]))
            for j in range(1, NJ):
                noff, nsz = nsplits[j]
                for k in range(KH):
                    head_dmas.append(
                        nc.sync.dma_start(xks[j][k][:], xt_d[:, k, noff:noff + nsz]))
            head_dmas.append(nc.sync.dma_start(w2f0[:], w2_d[:, 0:KFT, :]))
            for i in range(1, len(head_dmas)):
                add_dep_helper(head_dmas[i].ins, head_dmas[i - 1].ins,
                               sync=False, reason="head dma order")

            first_mm3 = None   # first mm3 matmul of previous ft
            for ft in range(NFT):
                if ft == 0:
                    w1f, v1f, w2f = w1f0, v1f0, w2f0
                else:
                    w1f = w1_pool.tile([P, KFT, KH, P], f16)
                    v1f = v1_pool.tile([P, KFT, KH, P], f16)
                    w2f = w2_pool.tile([P, KFT, H], f16)
                    # later tiles' weights ride the Scalar HWDGE queue and
                    # land during the previous tile's combine phase; gating
                    # the first DMA is enough — the Scalar engine issues the
                    # rest in program order behind it.
                    wdmas = []
                    for q in range(KFT):
                        wdmas.append(nc.scalar.dma_start(w1f[:, q], w1_d[:, ft, q]))
                        wdmas.append(nc.scalar.dma_start(v1f[:, q], v1_d[:, ft, q]))
                    nc.scalar.dma_start(w2f[:], w2_d[:, ft * KFT:(ft + 1) * KFT, :])
                    add_dep_helper(wdmas[0].ins, first_mm3.ins, sync=True,
                                   reason="stage weight prefetch")

                g = g_pool.tile([P, KFT, C], f16)
                for j, (noff, nsz) in enumerate(nsplits):
                    for q in range(KFT):
                        pa = pa_pool.tile([P, 512], f32)
                        pb = pb_pool.tile([P, 512], f32)
                        # interleave the two accumulations so each x chunk
                        # is consumed at two matmuls per DMA arrival
                        for k in range(KH):
                            nc.tensor.matmul(
                                pa[:, :nsz],
                                w1f[:, q, k],
                                xks[j][k][:],
                                start=(k == 0),
                                stop=(k == KH - 1),
                            )
                            nc.tensor.matmul(
                                pb[:, :nsz],
                                v1f[:, q, k],
                                xks[j][k][:],
                                start=(k == 0),
                                stop=(k == KH - 1),
                            )
                        sa = a_pool.tile([P, 512], f32)
                        nc.scalar.activation(
                            sa[:, :nsz], pa[:, :nsz],
                            mybir.ActivationFunctionType.Silu,
                        )
                        nc.vector.tensor_mul(
                            g[:, q, noff:noff + nsz], sa[:, :nsz], pb[:, :nsz]
                        )

                first_mm3 = None
                for mt in range(MT):
                    msz = csz[mt]
                    y16 = (y16_pool.tile([P, H], f16, name=f"y16_{mt}")
                           if ft == NFT - 1 else None)
                    for nh in range(NHT):
                        py = py_pool.tile([P, HT], f32)
                        for kk in range(KFT):
                            mm = nc.tensor.matmul(
                                py[:msz],
                                g[:, kk, mt * P:mt * P + msz],
                                w2f[:, kk, nh * HT:(nh + 1) * HT],
                                start=(kk == 0),
                                stop=(kk == KFT - 1),
                            )
                            if first_mm3 is None:
                                first_mm3 = mm
                        ysl = y_acc[:msz, mt, nh * HT:(nh + 1) * HT]
                        if ft == 0:
                            nc.vector.tensor_copy(ysl, py[:msz])
                        elif ft < NFT - 1:
                            nc.vector.tensor_add(ysl, ysl, py[:msz])
                        else:
                            # final tile: fold the last contribution and
                            # downcast to fp16 in one DVE pass
                            nc.vector.tensor_add(
                                y16[:msz, nh * HT:(nh + 1) * HT], ysl, py[:msz])
                            if mt == MT - 1:
                                # last row: drain each half right behind
                                # its add so the kernel tail is short
                                nc.sync.dma_start(
                                    y_d[:, mt, nh * HT:(nh + 1) * HT],
                                    y16[:, nh * HT:(nh + 1) * HT])
                    if ft == NFT - 1 and mt < MT - 1:
                        # whole-row eviction: 2KB runs on the idle Sync queue
                        nc.sync.dma_start(y_d[:, mt, :], y16[:])

    nc.compile()
    return nc


def _relayout_w1(w: np.ndarray) -> np.ndarray:
    # [F, H] -> [P, NFT, KFT, KH, P]: out[p, ft, q, k, m] = w[ft*FT+q*P+m, k*P+p]
    return np.ascontiguousarray(
        w.T.reshape(KH, P, NFT, KFT, P).transpose(1, 2, 3, 0, 4)).astype(np.float16)


def kernel(x, scores, expert_weights, top_experts, w1, v1, w2) -> np.ndarray:
    x = np.ascontiguousarray(np.asarray(x, dtype=np.float32))
    ew = np.asarray(expert_weights, dtype=np.float32)
    te = np.asarray(top_experts).astype(np.int64)
    w1 = np.asarray(w1, dtype=np.float32)
    v1 = np.asarray(v1, dtype=np.float32)
    w2 = np.asarray(w2, dtype=np.float32)

    t_num, h_num = x.shape
    e_num = w1.shape[0]

    gates = np.zeros((t_num, e_num), dtype=np.float32)
    np.add.at(gates, (np.arange(t_num)[:, None], te), ew)

    idxs = [np.flatnonzero((te == e).any(axis=1)) for e in range(e_num)]
    cmax = max(len(i) for i in idxs)
    C = max(512, ((cmax + 7) // 8) * 8)
    MT = (C + P - 1) // P

    if C not in _programs:
        _programs[C] = _build_program(C)
    nc = _programs[C]

    in_maps = []
    for e in range(e_num):
        idx = idxs[e]
        xe = np.zeros((C, h_num), np.float32)
        xe[:len(idx)] = x[idx]
        # [P, KH, C]: xt[p, k, c] = xe[c, k*P+p]
        xt = np.ascontiguousarray(
            xe.T.reshape(KH, P, C).transpose(1, 0, 2)).astype(np.float16)
        in_maps.append({
            "xt": xt,
            "w1t": _relayout_w1(w1[e]),
            "v1t": _relayout_w1(v1[e]),
            "w2": np.ascontiguousarray(w2[e].reshape(F // P, P, H).transpose(1, 0, 2)).astype(np.float16),
        })

    res = run_bass_kernel_spmd(nc, in_maps, core_ids=list(range(e_num)))

    out = np.zeros((t_num, h_num), np.float32)
    for e in range(e_num):
        idx = idxs[e]
        ye = res.results[e]["y"].astype(np.float32).transpose(1, 0, 2).reshape(MT * P, h_num)[:len(idx)]
        out[idx] += gates[idx, e:e + 1] * ye
    return out


# revision 9
# speedup vs baseline: 1.2088x; 1.2088x over previous
"""Dropless MoE GLU-MLP kernel for 8 Trainium2 NeuronCores.

Strategy: expert-parallel. Host computes the routing (gates + per-expert
token lists), gathers each expert's tokens, and ships one expert per core.
Each core runs a 3-matmul GLU MLP over its (padded) token batch with all
matmul operands in fp16 (e5m10; the PE multiplies at FP22 and accumulates
fp32, so end-to-end error is ~5e-4 — measured against an fp64 oracle):

    AT = w1e @ Xe.T          [F, C]   (stationary = w1t chunks, moving = Xt)
    BT = v1e @ Xe.T          [F, C]
    GT = silu(AT) * BT       [F, C]   (ACT silu + DVE mul, PSUM-evicted)
    Y  = GT.T @ w2e          [C, H]   (stationary = GT chunks, moving = w2)

All matmuls use moving dim <=512 (one PSUM bank); at 512 they profile at
the 215.8 ns/MM issue floor and the body is PE-bound end to end, so the
optimization targets are the head (first real matmul issuing early, PE
never starved) and the tail (drain Y fast):

 - Per-core HBM read bandwidth is ~300 GB/s TOTAL across both HWDGE
   queues, so the head is fed by a single Sync-queue DMA stream whose
   issue order exactly matches PE consumption order: w1/v1 block 0, the
   eight j0-half x chunks (131 KB each, matching the 432 ns/chunk PE
   consumption rate), then the later w1/v1 blocks, the j1-half x chunks,
   and ft0's w2. No semaphore gating needed — queue order IS the
   bandwidth allocation. The first real matmul starts ~10 us in (vs ~16
   us for a monolithic xt load) and warmup matmuls keep the PE clock
   ramp continuous through the handoff (a PE idle gap costs ~6 us of
   half-rate re-ramp on top of the gap itself).
 - the mm1/mm2 k-loop interleaves the w1(pa) and v1(pb) accumulations so
   each x chunk is consumed at two matmuls per arrival.
 - later F-tiles' weights ride the Scalar HWDGE queue (its transfers land
   during the previous tile's combine phase when the Sync queue is idle),
   gated behind the previous tile's first combine matmul.
 - C pads the max per-expert token count to a multiple of 8 (not 128),
   shaving dead columns off mm1/mm2's moving dim; mm3's stationary
   chunks handle the ragged tail (<=128 wide costs the same instruction).
 - Y accumulates fp32 in SBUF across F-tiles and is emitted fp16 during
   the last F-tile as whole [P, H] rows (2 KB runs); the last row is
   split in half so the final DMA chases the final DVE add by ~0.4 us.

Host scatter-adds gate-scaled Y back to the full output.
"""

import numpy as np

import concourse.bass as bass
import concourse.tile as tile
from concourse import bacc, mybir
from concourse.bass_utils import run_bass_kernel_spmd
from concourse.tile import add_dep_helper

T, H, F, E, TOPK = 4096, 1024, 4096, 8, 2
P = 128
KH = H // P            # 8 k-chunks over the H contraction
FT = 512               # F tile width
NFT = F // FT          # 8 F tiles
KFT = FT // P          # 4 k-chunks per F tile in the combine matmul
HT = 512               # moving tile of H in the combine matmul
NHT = H // HT          # 2

_programs: dict[int, object] = {}


def _ntile_splits(C: int) -> list[tuple[int, int]]:
    """Split C into moving-dim tiles of <=512."""
    splits, off, rem = [], 0, C
    while rem > 0:
        take = min(512, rem)
        splits.append((off, take))
        off += take
        rem -= take
    return splits


def _build_program(C: int):
    f32 = mybir.dt.float32
    f16 = mybir.dt.float16
    MT = (C + P - 1) // P                # mm3 stationary chunks
    csz = [min(P, C - mt * P) for mt in range(MT)]
    nsplits = _ntile_splits(C)
    NJ = len(nsplits)

    nc = bacc.Bacc("TRN2", target_bir_lowering=False, debug=False, num_devices=E)
    # xt: [P, KH, C] — per-(k, j-half) slices are contiguous >=1KB runs
    xt_d = nc.dram_tensor("xt", [P, KH, C], f16, kind="ExternalInput").ap()
    # w1t/v1t: [P, NFT, KFT, KH, P] — one ft/block slice is a 2KB run
    w1_d = nc.dram_tensor("w1t", [P, NFT, KFT, KH, P], f16, kind="ExternalInput").ap()
    v1_d = nc.dram_tensor("v1t", [P, NFT, KFT, KH, P], f16, kind="ExternalInput").ap()
    w2_d = nc.dram_tensor("w2", [P, F // P, H], f16, kind="ExternalInput").ap()
    y_d = nc.dram_tensor("y", [P, MT, H], f16, kind="ExternalOutput").ap()

    with tile.TileContext(nc) as tc:
        with (
            tc.tile_pool(name="xk", bufs=1) as xk_pool,
            tc.tile_pool(name="yacc", bufs=1) as y_pool,
            tc.tile_pool(name="y16", bufs=2) as y16_pool,
            tc.tile_pool(name="w1f", bufs=2) as w1_pool,
            tc.tile_pool(name="v1f", bufs=2) as v1_pool,
            tc.tile_pool(name="w2f", bufs=2) as w2_pool,
            tc.tile_pool(name="gt", bufs=2) as g_pool,
            tc.tile_pool(name="sa", bufs=2) as a_pool,
            tc.tile_pool(name="wu", bufs=1) as wu_pool,
            tc.tile_pool(name="pa", bufs=2, space="PSUM") as pa_pool,
            tc.tile_pool(name="pb", bufs=2, space="PSUM") as pb_pool,
            tc.tile_pool(name="py", bufs=4, space="PSUM") as py_pool,
        ):
            # PE warmup during the initial DMA window: junk matmuls flip the
            # HAM clock gate to 8/8 and start the pstate ramp before the
            # first real matmul issues (~10.1us in).
            wu = wu_pool.tile([P, 512], f16)
            nc.vector.memset(wu[:], 0.0)
            wps = [pa_pool.tile([P, 512], f32, tag="pa", name="wp_a"),
                   pb_pool.tile([P, 512], f32, tag="pb", name="wp_b")]
            for i in range(7):
                nc.tensor.matmul(wps[i % 2][:], wu[:, :P], wu[:],
                                 start=True, stop=True)

            y_acc = y_pool.tile([P, MT, H], f32)

            # ft0 weight tiles (loaded on the Sync queue, interleaved with x)
            w1f0 = w1_pool.tile([P, KFT, KH, P], f16, name="w1f0")
            v1f0 = v1_pool.tile([P, KFT, KH, P], f16, name="v1f0")
            w2f0 = w2_pool.tile([P, KFT, H], f16, name="w2f0")

            # x chunk tiles, split by (j-half, k)
            xks = [[xk_pool.tile([P, nsz], f16, name=f"xk{j}_{k}",
                                 tag=f"xk{j}_{k}") for k in range(KH)]
                   for j, (noff, nsz) in enumerate(nsplits)]

            # ---- head DMA stream ----
            # The tile scheduler freely reorders independent DMAs across the
            # two HWDGE queues (hoisting later weight loads ahead of the x
            # chunks starves the PE — per-core HBM read BW is ~300GB/s total
            # shared by both queues). Leave only the block-0 weights and the
            # j0-half x chunks ungated — everything the PE touches in its
            # first ~3.5us — and hold each later group behind a PE matmul
            # whose firing time leaves enough slack for the ~1.4us
            # release-to-first-byte latency. q0_gate_mms is filled in by the
            # mm1/mm2 loop below (ft0 j0 quarter-q first matmuls).
            noff0, nsz0 = nsplits[0]
            nc.sync.dma_start(w1f0[:, 0], w1_d[:, 0, 0])
            nc.sync.dma_start(v1f0[:, 0], v1_d[:, 0, 0])
            for k in range(KH):
                nc.sync.dma_start(xks[0][k][:], xt_d[:, k, noff0:noff0 + nsz0])
            gated_head: list[tuple[object, int]] = []  # (dma, gate quarter idx)
            for q in range(1, KFT):
                gated_head.append((nc.sync.dma_start(w1f0[:, q], w1_d[:, 0, q]), q - 1))
                gated_head.append((nc.sync.dma_start(v1f0[:, q], v1_d[:, 0, q]), q - 1))
            for j in range(1, NJ):
                noff, nsz = nsplits[j]
                for k in range(KH):
                    gated_head.append(
                        (nc.sync.dma_start(xks[j][k][:], xt_d[:, k, noff:noff + nsz]), 1))
            gated_head.append((nc.sync.dma_start(w2f0[:], w2_d[:, 0:KFT, :]), 2))

            first_mm3 = None   # first mm3 matmul of previous ft
            for ft in range(NFT):
                if ft == 0:
                    w1f, v1f, w2f = w1f0, v1f0, w2f0
                else:
                    w1f = w1_pool.tile([P, KFT, KH, P], f16)
                    v1f = v1_pool.tile([P, KFT, KH, P], f16)
                    w2f = w2_pool.tile([P, KFT, H], f16)
                    # later tiles' weights ride the Scalar HWDGE queue and
                    # land during the previous tile's combine phase. Gate
                    # EVERY dma — ungated followers get hoisted into the
                    # head window by the tile scheduler.
                    wdmas = []
                    for q in range(KFT):
                        wdmas.append(nc.scalar.dma_start(w1f[:, q], w1_d[:, ft, q]))
                        wdmas.append(nc.scalar.dma_start(v1f[:, q], v1_d[:, ft, q]))
                    wdmas.append(
                        nc.scalar.dma_start(w2f[:], w2_d[:, ft * KFT:(ft + 1) * KFT, :]))
                    for wd in wdmas:
                        add_dep_helper(wd.ins, first_mm3.ins, sync=True,
                                       reason="stage weight prefetch")

                g = g_pool.tile([P, KFT, C], f16)
                q_first_mm: list = [None] * KFT
                for j, (noff, nsz) in enumerate(nsplits):
                    for q in range(KFT):
                        pa = pa_pool.tile([P, 512], f32)
                        pb = pb_pool.tile([P, 512], f32)
                        # interleave the two accumulations so each x chunk
                        # is consumed at two matmuls per DMA arrival
                        for k in range(KH):
                            mm = nc.tensor.matmul(
                                pa[:, :nsz],
                                w1f[:, q, k],
                                xks[j][k][:],
                                start=(k == 0),
                                stop=(k == KH - 1),
                            )
                            if j == 0 and q_first_mm[q] is None:
                                q_first_mm[q] = mm
                            nc.tensor.matmul(
                                pb[:, :nsz],
                                v1f[:, q, k],
                                xks[j][k][:],
                                start=(k == 0),
                                stop=(k == KH - 1),
                            )
                        sa = a_pool.tile([P, 512], f32)
                        nc.scalar.activation(
                            sa[:, :nsz], pa[:, :nsz],
                            mybir.ActivationFunctionType.Silu,
                        )
                        nc.vector.tensor_mul(
                            g[:, q, noff:noff + nsz], sa[:, :nsz], pb[:, :nsz]
                        )
                if ft == 0:
                    # release the staged head loads against j0-quarter
                    # progress (gate EVERY dma in a group — ungated
                    # followers get hoisted by the scheduler)
                    for dma, qgate in gated_head:
                        add_dep_helper(dma.ins, q_first_mm[qgate].ins,
                                       sync=True, reason=f"stage head q{qgate}")

                first_mm3 = None
                for mt in range(MT):
                    msz = csz[mt]
                    y16 = (y16_pool.tile([P, H], f16, name=f"y16_{mt}")
                           if ft == NFT - 1 else None)
                    for nh in range(NHT):
                        py = py_pool.tile([P, HT], f32)
                        for kk in range(KFT):
                            mm = nc.tensor.matmul(
                                py[:msz],
                                g[:, kk, mt * P:mt * P + msz],
                                w2f[:, kk, nh * HT:(nh + 1) * HT],
                                start=(kk == 0),
                                stop=(kk == KFT - 1),
                            )
                            if first_mm3 is None:
                                first_mm3 = mm
                        ysl = y_acc[:msz, mt, nh * HT:(nh + 1) * HT]
                        if ft == 0:
                            nc.vector.tensor_copy(ysl, py[:msz])
                        elif ft < NFT - 1:
                            nc.vector.tensor_add(ysl, ysl, py[:msz])
                        else:
                            # final tile: fold the last contribution and
                            # downcast to fp16 in one DVE pass
                            nc.vector.tensor_add(
                                y16[:msz, nh * HT:(nh + 1) * HT], ysl, py[:msz])
                            if mt == MT - 1:
                                # last row: drain each half right behind
                                # its add so the kernel tail is short
                                nc.sync.dma_start(
                                    y_d[:, mt, nh * HT:(nh + 1) * HT],
                                    y16[:, nh * HT:(nh + 1) * HT])
                    if ft == NFT - 1 and mt < MT - 1:
                        # whole-row eviction: 2KB runs on the idle Sync queue
                        nc.sync.dma_start(y_d[:, mt, :], y16[:])

    nc.compile()
    return nc


def _relayout_w1(w: np.ndarray) -> np.ndarray:
    # [F, H] -> [P, NFT, KFT, KH, P]: out[p, ft, q, k, m] = w[ft*FT+q*P+m, k*P+p]
    return np.ascontiguousarray(
        w.T.reshape(KH, P, NFT, KFT, P).transpose(1, 2, 3, 0, 4)).astype(np.float16)


def kernel(x, scores, expert_weights, top_experts, w1, v1, w2) -> np.ndarray:
    x = np.ascontiguousarray(np.asarray(x, dtype=np.float32))
    ew = np.asarray(expert_weights, dtype=np.float32)
    te = np.asarray(top_experts).astype(np.int64)
    w1 = np.asarray(w1, dtype=np.float32)
    v1 = np.asarray(v1, dtype=np.float32)
    w2 = np.asarray(w2, dtype=np.float32)

    t_num, h_num = x.shape
    e_num = w1.shape[0]

    gates = np.zeros((t_num, e_num), dtype=np.float32)
    np.add.at(gates, (np.arange(t_num)[:, None], te), ew)

    idxs = [np.flatnonzero((te == e).any(axis=1)) for e in range(e_num)]
    cmax = max(len(i) for i in idxs)
    C = max(512, ((cmax + 7) // 8) * 8)
    MT = (C + P - 1) // P

    if C not in _programs:
        _programs[C] = _build_program(C)
    nc = _programs[C]

    in_maps = []
    for e in range(e_num):
        idx = idxs[e]
        xe = np.zeros((C, h_num), np.float32)
        xe[:len(idx)] = x[idx]
        # [P, KH, C]: xt[p, k, c] = xe[c, k*P+p]
        xt = np.ascontiguousarray(
            xe.T.reshape(KH, P, C).transpose(1, 0, 2)).astype(np.float16)
        in_maps.append({
            "xt": xt,
            "w1t": _relayout_w1(w1[e]),
            "v1t": _relayout_w1(v1[e]),
            "w2": np.ascontiguousarray(w2[e].reshape(F // P, P, H).transpose(1, 0, 2)).astype(np.float16),
        })

    res = run_bass_kernel_spmd(nc, in_maps, core_ids=list(range(e_num)))

    out = np.zeros((t_num, h_num), np.float32)
    for e in range(e_num):
        idx = idxs[e]
        ye = res.results[e]["y"].astype(np.float32).transpose(1, 0, 2).reshape(MT * P, h_num)[:len(idx)]
        out[idx] += gates[idx, e:e + 1] * ye
    return out


# revision 11
# speedup vs baseline: 1.2198x; 1.0091x over previous
"""Dropless MoE GLU-MLP kernel for 8 Trainium2 NeuronCores.

Strategy: expert-parallel. Host computes the routing (gates + per-expert
token lists), gathers each expert's tokens, and ships one expert per core.
Each core runs a 3-matmul GLU MLP over its (padded) token batch with all
matmul operands in fp16 (e5m10; the PE multiplies at FP22 and accumulates
fp32, so end-to-end error is ~5e-4 — measured against an fp64 oracle):

    AT = w1e @ Xe.T          [F, C]   (stationary = w1t chunks, moving = Xt)
    BT = v1e @ Xe.T          [F, C]
    GT = silu(AT) * BT       [F, C]   (ACT silu + DVE mul, PSUM-evicted)
    Y  = GT.T @ w2e          [C, H]   (stationary = GT chunks, moving = w2)

All matmuls use moving dim <=512 (one PSUM bank); at 512 they profile at
the 215.8 ns/MM issue floor and the body is PE-bound end to end, so the
optimization targets are the head (first real matmul issuing early, PE
never starved) and the tail (drain Y fast):

 - Per-core HBM read bandwidth is ~300 GB/s TOTAL across both HWDGE
   queues, so the head is fed by a single Sync-queue DMA stream whose
   issue order exactly matches PE consumption order: w1/v1 block 0, the
   eight j0-half x chunks (131 KB each, matching the 432 ns/chunk PE
   consumption rate), then the later w1/v1 blocks, the j1-half x chunks,
   and ft0's w2. No semaphore gating needed — queue order IS the
   bandwidth allocation. The first real matmul starts ~10 us in (vs ~16
   us for a monolithic xt load) and warmup matmuls keep the PE clock
   ramp continuous through the handoff (a PE idle gap costs ~6 us of
   half-rate re-ramp on top of the gap itself).
 - the mm1/mm2 k-loop interleaves the w1(pa) and v1(pb) accumulations so
   each x chunk is consumed at two matmuls per arrival.
 - later F-tiles' weights ride the Scalar HWDGE queue (its transfers land
   during the previous tile's combine phase when the Sync queue is idle),
   gated behind the previous tile's first combine matmul.
 - C pads the max per-expert token count to a multiple of 8 (not 128),
   shaving dead columns off mm1/mm2's moving dim; mm3's stationary
   chunks handle the ragged tail (<=128 wide costs the same instruction).
 - Y accumulates fp32 in SBUF across F-tiles and is emitted fp16 during
   the last F-tile as whole [P, H] rows (2 KB runs); the last row is
   split in half so the final DMA chases the final DVE add by ~0.4 us.

Host scatter-adds gate-scaled Y back to the full output.
"""

import numpy as np

import concourse.bass as bass
import concourse.tile as tile
from concourse import bacc, mybir
from concourse.bass_utils import run_bass_kernel_spmd
from concourse.tile import add_dep_helper

T, H, F, E, TOPK = 4096, 1024, 4096, 8, 2
P = 128
KH = H // P            # 8 k-chunks over the H contraction
FT = 512               # F tile width
NFT = F // FT          # 8 F tiles
KFT = FT // P          # 4 k-chunks per F tile in the combine matmul
HT = 512               # moving tile of H in the combine matmul
NHT = H // HT          # 2

_programs: dict[int, object] = {}


def _ntile_splits(C: int) -> list[tuple[int, int]]:
    """Split C into moving-dim tiles of <=512."""
    splits, off, rem = [], 0, C
    while rem > 0:
        take = min(512, rem)
        splits.append((off, take))
        off += take
        rem -= take
    return splits


def _build_program(C: int):
    f32 = mybir.dt.float32
    f16 = mybir.dt.float16
    MT = (C + P - 1) // P                # mm3 stationary chunks
    csz = [min(P, C - mt * P) for mt in range(MT)]
    nsplits = _ntile_splits(C)
    NJ = len(nsplits)

    nc = bacc.Bacc("TRN2", target_bir_lowering=False, debug=False, num_devices=E)
    # xt: [P, KH, C] — per-(k, j-half) slices are contiguous >=1KB runs
    xt_d = nc.dram_tensor("xt", [P, KH, C], f16, kind="ExternalInput").ap()
    # w1t/v1t: [P, NFT, KFT, KH, P] — one ft/block slice is a 2KB run
    w1_d = nc.dram_tensor("w1t", [P, NFT, KFT, KH, P], f16, kind="ExternalInput").ap()
    v1_d = nc.dram_tensor("v1t", [P, NFT, KFT, KH, P], f16, kind="ExternalInput").ap()
    w2_d = nc.dram_tensor("w2", [P, F // P, H], f16, kind="ExternalInput").ap()
    y_d = nc.dram_tensor("y", [P, MT, H], f16, kind="ExternalOutput").ap()

    with tile.TileContext(nc) as tc:
        with (
            tc.tile_pool(name="xk", bufs=1) as xk_pool,
            tc.tile_pool(name="yacc", bufs=1) as y_pool,
            tc.tile_pool(name="y16", bufs=2) as y16_pool,
            tc.tile_pool(name="w1f", bufs=2) as w1_pool,
            tc.tile_pool(name="v1f", bufs=2) as v1_pool,
            tc.tile_pool(name="w2f", bufs=2) as w2_pool,
            tc.tile_pool(name="gt", bufs=2) as g_pool,
            tc.tile_pool(name="sa", bufs=2) as a_pool,
            tc.tile_pool(name="wu", bufs=1) as wu_pool,
            tc.tile_pool(name="pa", bufs=2, space="PSUM") as pa_pool,
            tc.tile_pool(name="pb", bufs=2, space="PSUM") as pb_pool,
            tc.tile_pool(name="py", bufs=4, space="PSUM") as py_pool,
        ):
            # PE warmup during the initial DMA window: junk matmuls flip the
            # HAM clock gate to 8/8 and start the pstate ramp before the
            # first real matmul issues (~10.1us in).
            wu = wu_pool.tile([P, 512], f16)
            nc.vector.memset(wu[:], 0.0)
            wps = [pa_pool.tile([P, 512], f32, tag="pa", name="wp_a"),
                   pb_pool.tile([P, 512], f32, tag="pb", name="wp_b")]
            for i in range(10):
                nc.tensor.matmul(wps[i % 2][:], wu[:, :P], wu[:],
                                 start=True, stop=True)

            y_acc = y_pool.tile([P, MT, H], f32)

            # ft0 weight tiles (loaded on the Sync queue, interleaved with x)
            w1f0 = w1_pool.tile([P, KFT, KH, P], f16, name="w1f0")
            v1f0 = v1_pool.tile([P, KFT, KH, P], f16, name="v1f0")
            w2f0 = w2_pool.tile([P, KFT, H], f16, name="w2f0")

            # x chunk tiles, split by (j-half, k)
            xks = [[xk_pool.tile([P, nsz], f16, name=f"xk{j}_{k}",
                                 tag=f"xk{j}_{k}") for k in range(KH)]
                   for j, (noff, nsz) in enumerate(nsplits)]

            # ---- head DMA stream ----
            # The tile scheduler freely reorders independent DMAs across the
            # two HWDGE queues (hoisting later weight loads ahead of the x
            # chunks starves the PE — per-core HBM read BW is ~300GB/s total
            # shared by both queues). Leave only the block-0 weights and the
            # j0-half x chunks ungated — everything the PE touches in its
            # first ~3.5us — and hold each later group behind a PE matmul
            # whose firing time leaves enough slack for the ~1.4us
            # release-to-first-byte latency. q0_gate_mms is filled in by the
            # mm1/mm2 loop below (ft0 j0 quarter-q first matmuls).
            noff0, nsz0 = nsplits[0]
            nc.sync.dma_start(w1f0[:, 0], w1_d[:, 0, 0])
            nc.sync.dma_start(v1f0[:, 0], v1_d[:, 0, 0])
            for k in range(KH):
                nc.sync.dma_start(xks[0][k][:], xt_d[:, k, noff0:noff0 + nsz0])
            gated_head: list[tuple[object, int]] = []  # (dma, gate quarter idx)
            for q in range(1, KFT):
                qg = 0 if q < KFT - 1 else 1
                gated_head.append((nc.sync.dma_start(w1f0[:, q], w1_d[:, 0, q]), qg))
                gated_head.append((nc.sync.dma_start(v1f0[:, q], v1_d[:, 0, q]), qg))
            for j in range(1, NJ):
                noff, nsz = nsplits[j]
                for k in range(KH):
                    gated_head.append(
                        (nc.sync.dma_start(xks[j][k][:], xt_d[:, k, noff:noff + nsz]), 1))
            gated_head.append((nc.sync.dma_start(w2f0[:], w2_d[:, 0:KFT, :]), 2))

            first_mm3 = None   # first mm3 matmul of previous ft
            for ft in range(NFT):
                if ft == 0:
                    w1f, v1f, w2f = w1f0, v1f0, w2f0
                else:
                    w1f = w1_pool.tile([P, KFT, KH, P], f16)
                    v1f = v1_pool.tile([P, KFT, KH, P], f16)
                    w2f = w2_pool.tile([P, KFT, H], f16)
                    # later tiles' weights ride the Scalar HWDGE queue and
                    # land during the previous tile's combine phase. Gate
                    # EVERY dma — ungated followers get hoisted into the
                    # head window by the tile scheduler.
                    wdmas = []
                    for q in range(KFT):
                        wdmas.append(nc.scalar.dma_start(w1f[:, q], w1_d[:, ft, q]))
                        wdmas.append(nc.scalar.dma_start(v1f[:, q], v1_d[:, ft, q]))
                    wdmas.append(
                        nc.scalar.dma_start(w2f[:], w2_d[:, ft * KFT:(ft + 1) * KFT, :]))
                    for wd in wdmas:
                        add_dep_helper(wd.ins, first_mm3.ins, sync=True,
                                       reason="stage weight prefetch")

                g = g_pool.tile([P, KFT, C], f16)
                q_first_mm: list = [None] * KFT
                for j, (noff, nsz) in enumerate(nsplits):
                    for q in range(KFT):
                        pa = pa_pool.tile([P, 512], f32)
                        pb = pb_pool.tile([P, 512], f32)
                        # interleave the two accumulations so each x chunk
                        # is consumed at two matmuls per DMA arrival
                        for k in range(KH):
                            mm = nc.tensor.matmul(
                                pa[:, :nsz],
                                w1f[:, q, k],
                                xks[j][k][:],
                                start=(k == 0),
                                stop=(k == KH - 1),
                            )
                            if j == 0 and q_first_mm[q] is None:
                                q_first_mm[q] = mm
                            nc.tensor.matmul(
                                pb[:, :nsz],
                                v1f[:, q, k],
                                xks[j][k][:],
                                start=(k == 0),
                                stop=(k == KH - 1),
                            )
                        sa = a_pool.tile([P, 512], f32)
                        nc.scalar.activation(
                            sa[:, :nsz], pa[:, :nsz],
                            mybir.ActivationFunctionType.Silu,
                        )
                        nc.vector.tensor_mul(
                            g[:, q, noff:noff + nsz], sa[:, :nsz], pb[:, :nsz]
                        )
                if ft == 0:
                    # release the staged head loads against j0-quarter
                    # progress (gate EVERY dma in a group — ungated
                    # followers get hoisted by the scheduler)
                    for dma, qgate in gated_head:
                        add_dep_helper(dma.ins, q_first_mm[qgate].ins,
                                       sync=True, reason=f"stage head q{qgate}")

                first_mm3 = None
                for mt in range(MT):
                    msz = csz[mt]
                    y16 = (y16_pool.tile([P, H], f16, name=f"y16_{mt}")
                           if ft == NFT - 1 else None)
                    for nh in range(NHT):
                        py = py_pool.tile([P, HT], f32)
                        for kk in range(KFT):
                            mm = nc.tensor.matmul(
                                py[:msz],
                                g[:, kk, mt * P:mt * P + msz],
                                w2f[:, kk, nh * HT:(nh + 1) * HT],
                                start=(kk == 0),
                                stop=(kk == KFT - 1),
                            )
                            if first_mm3 is None:
                                first_mm3 = mm
                        ysl = y_acc[:msz, mt, nh * HT:(nh + 1) * HT]
                        if ft == 0:
                            nc.vector.tensor_copy(ysl, py[:msz])
                        elif ft < NFT - 1:
                            nc.vector.tensor_add(ysl, ysl, py[:msz])
                        else:
                            # final tile: fold the last contribution and
                            # downcast to fp16 in one DVE pass
                            nc.vector.tensor_add(
                                y16[:msz, nh * HT:(nh + 1) * HT], ysl, py[:msz])
                            if mt == MT - 1:
                                # last row: drain each half right behind
                                # its add so the kernel tail is short
                                nc.sync.dma_start(
                                    y_d[:, mt, nh * HT:(nh + 1) * HT],
                                    y16[:, nh * HT:(nh + 1) * HT])
                    if ft == NFT - 1 and mt < MT - 1:
                        # whole-row eviction: 2KB runs on the idle Sync queue
                        nc.sync.dma_start(y_d[:, mt, :], y16[:])

    nc.compile()
    return nc


def _relayout_w1(w: np.ndarray) -> np.ndarray:
    # [F, H] -> [P, NFT, KFT, KH, P]: out[p, ft, q, k, m] = w[ft*FT+q*P+m, k*P+p]
    return np.ascontiguousarray(
        w.T.reshape(KH, P, NFT, KFT, P).transpose(1, 2, 3, 0, 4)).astype(np.float16)


def kernel(x, scores, expert_weights, top_experts, w1, v1, w2) -> np.ndarray:
    x = np.ascontiguousarray(np.asarray(x, dtype=np.float32))
    ew = np.asarray(expert_weights, dtype=np.float32)
    te = np.asarray(top_experts).astype(np.int64)
    w1 = np.asarray(w1, dtype=np.float32)
    v1 = np.asarray(v1, dtype=np.float32)
    w2 = np.asarray(w2, dtype=np.float32)

    t_num, h_num = x.shape
    e_num = w1.shape[0]

    gates = np.zeros((t_num, e_num), dtype=np.float32)
    np.add.at(gates, (np.arange(t_num)[:, None], te), ew)

    idxs = [np.flatnonzero((te == e).any(axis=1)) for e in range(e_num)]
    cmax = max(len(i) for i in idxs)
    C = max(512, ((cmax + 7) // 8) * 8)
    MT = (C + P - 1) // P

    if C not in _programs:
        _programs[C] = _build_program(C)
    nc = _programs[C]

    in_maps = []
    for e in range(e_num):
        idx = idxs[e]
        xe = np.zeros((C, h_num), np.float32)
        xe[:len(idx)] = x[idx]
        # [P, KH, C]: xt[p, k, c] = xe[c, k*P+p]
        xt = np.ascontiguousarray(
            xe.T.reshape(KH, P, C).transpose(1, 0, 2)).astype(np.float16)
        in_maps.append({
            "xt": xt,
            "w1t": _relayout_w1(w1[e]),
            "v1t": _relayout_w1(v1[e]),
            "w2": np.ascontiguousarray(w2[e].reshape(F // P, P, H).transpose(1, 0, 2)).astype(np.float16),
        })

    res = run_bass_kernel_spmd(nc, in_maps, core_ids=list(range(e_num)))

    out = np.zeros((t_num, h_num), np.float32)
    for e in range(e_num):
        idx = idxs[e]
        ye = res.results[e]["y"].astype(np.float32).transpose(1, 0, 2).reshape(MT * P, h_num)[:len(idx)]
        out[idx] += gates[idx, e:e + 1] * ye
    return out
